# revision 11
# baseline (speedup 1.0000x reference)
"""Bidirectional cross-attention kernel for Trainium2, 8 NeuronCores.

Sharding: core c handles batch b = c//2 and head-group g = c%2 (4 of the 8
heads).  Each core computes q/k/v projections for its heads, both attention
directions (seq1->seq2 and seq2->seq1), and a partial output projection over
its head-group's hid columns.  The host sums the two head-group partials per
batch and adds bias + residual.

Layouts on device (per core):
  x1 [C1=256, N1=4096], x2 [C2=512, N2=1024]  (channels-major, bf16)
  q/k [D-pair=128, N] with head pair (2h, 2h+1) stacked on partitions
  vT_ext [128 kpos-tile, ntiles, 130] = [v_h0 | ones | v_h1 | ones]
  Attention processes 512 queries at a time.  Per key-tile m (128 kpos):
  S^T for both heads of a pair goes into one [128, 2, 512] PSUM tile
  (row-group-paired matmuls at partition bases 0/64), one exp on ACT
  covers both heads -> attn^T bf16 in SBUF, PV matmuls accumulate
  [65, 512] per head (64 v-dims + ones-row = softmax denominator).
  The two head-pairs are interleaved inside the m loop so one pair's
  normalize tail hides under the other pair's matmuls.
  Normalization: reciprocal of the denominator row,
  partition-broadcast via a DRAM bounce, multiply on DVE while copying
  PSUM->SBUF (bf16).  Output projection runs per 512-query block as PE
  gap filler.
"""

import sys

for _p in ("/opt/trn_rl_repo",):
    if _p not in sys.path:
        sys.path.insert(0, _p)

import numpy as np
import ml_dtypes

import concourse.bass as bass
import concourse.tile as tile
from concourse import bacc, mybir
from concourse.bass_utils import run_bass_kernel_spmd

BF = mybir.dt.bfloat16
F32 = mybir.dt.float32
Exp = mybir.ActivationFunctionType.Exp

HEADS = 8
D = 64
SCALE = D ** -0.5
C1, N1 = 256, 4096
C2, N2 = 512, 1024
NPAIR = 2       # head pairs per core
N_CORES = 8
QW = 512        # query block width

TRACE = False
TMPDIR = None
LAST = {}

_PROGRAM = None


def _dram_bcast(ap_1xN, nparts):
    """Partition-broadcast AP for a DRAM source row."""
    return bass.AP(
        tensor=ap_1xN.tensor,
        offset=ap_1xN.offset,
        ap=[[0, nparts]] + [list(x) for x in ap_1xN.ap[1:]],
    )


def _build_program():
    nc = bacc.Bacc("TRN2", target_bir_lowering=False, debug=True)

    x1_d = nc.declare_dram_parameter("x1", [C1, N1], BF, isOutput=False)
    x2_d = nc.declare_dram_parameter("x2", [C2, N2], BF, isOutput=False)
    wq1_d = nc.declare_dram_parameter("wq1", [C1, 256], BF, isOutput=False)
    wk1_d = nc.declare_dram_parameter("wk1", [C1, 256], BF, isOutput=False)
    wv1_d = nc.declare_dram_parameter("wv1", [C1, 256], BF, isOutput=False)
    wq2_d = nc.declare_dram_parameter("wq2", [C2, 256], BF, isOutput=False)
    wk2_d = nc.declare_dram_parameter("wk2", [C2, 256], BF, isOutput=False)
    wv2_d = nc.declare_dram_parameter("wv2", [C2, 256], BF, isOutput=False)
    wo1_d = nc.declare_dram_parameter("wo1", [256, C1], BF, isOutput=False)
    wo2_d = nc.declare_dram_parameter("wo2", [256, C2], BF, isOutput=False)
    o1p_d = nc.declare_dram_parameter("o1p", [C1, N1], F32, isOutput=True)
    o2p_d = nc.declare_dram_parameter("o2p", [C2, N2], F32, isOutput=True)

    with tile.TileContext(nc) as tc:
        with tc.tile_pool(name="const", bufs=1) as cst, \
             tc.tile_pool(name="ps", bufs=2, space="PSUM") as psp, \
             tc.tile_pool(name="att", bufs=8) as att, \
             tc.tile_pool(name="sc", bufs=4) as sc, \
             tc.tile_pool(name="dbounce", bufs=6, space="DRAM") as dbp:

            # ---- load inputs (small x2 first so compute starts early) ----
            x2 = []
            for c in range(4):
                t = cst.tile([128, N2], BF, tag=f"x2_{c}", name=f"x2_{c}")
                nc.sync.dma_start(out=t, in_=x2_d[:][128 * c:128 * (c + 1), :])
                x2.append(t)
            x1 = []
            for c in range(2):
                t = cst.tile([128, N1], BF, tag=f"x1_{c}", name=f"x1_{c}")
                nc.sync.dma_start(out=t, in_=x1_d[:][128 * c:128 * (c + 1), :])
                x1.append(t)

            def load_w(d, kchunks, cols, name):
                ts = []
                for c in range(kchunks):
                    t = cst.tile([128, cols], BF,
                                 tag=f"{name}_{c}", name=f"{name}_{c}")
                    nc.sync.dma_start(out=t, in_=d[:][128 * c:128 * (c + 1), :])
                    ts.append(t)
                return ts

            wk2 = load_w(wk2_d, 4, 256, "wk2")
            wv2 = load_w(wv2_d, 4, 256, "wv2")
            wq1 = load_w(wq1_d, 2, 256, "wq1")
            wk1 = load_w(wk1_d, 2, 256, "wk1")
            wv1 = load_w(wv1_d, 2, 256, "wv1")
            wq2 = load_w(wq2_d, 4, 256, "wq2")
            wo1 = load_w(wo1_d, 2, C1, "wo1")
            wo2 = load_w(wo2_d, 2, C2, "wo2")

            # ---- projections: q/k in [D-pair, N] layout ----
            def proj_qk(wt, xt, n, name):
                outs = []
                kchunks = len(wt)
                for p in range(NPAIR):
                    dst = cst.tile([128, n], BF,
                                   tag=f"{name}_{p}", name=f"{name}_{p}")
                    for nt in range(n // 1024):
                        pm = psp.tile([128, 1024], F32, tag="st", name="st")
                        for c in range(kchunks):
                            for h2 in range(2):
                                nc.tensor.matmul(
                                    pm[:, 512 * h2:512 * (h2 + 1)],
                                    wt[c][:, 128 * p:128 * (p + 1)],
                                    xt[c][:, 1024 * nt + 512 * h2:
                                          1024 * nt + 512 * (h2 + 1)],
                                    start=(c == 0), stop=(c == kchunks - 1),
                                )
                        nc.vector.tensor_copy(
                            dst[:, 1024 * nt:1024 * (nt + 1)], pm)
                    outs.append(dst)
                return outs

            # ---- projections: v^T extended with ones column ----
            def proj_vt(wt, xt, n, name):
                ntiles = n // 128
                kchunks = len(wt)
                outs = []
                for p in range(NPAIR):
                    t = cst.tile([128, ntiles, 130], BF,
                                 tag=f"{name}_{p}", name=f"{name}_{p}")
                    nc.vector.memset(t[:, :, 64:65], 1.0)
                    nc.vector.memset(t[:, :, 129:130], 1.0)
                    outs.append(t)
                for ti in range(ntiles):
                    pm = psp.tile([128, 256], F32, tag="pv", name="pv", bufs=4)
                    for c in range(kchunks):
                        nc.tensor.matmul(
                            pm, xt[c][:, 128 * ti:128 * (ti + 1)], wt[c],
                            start=(c == 0), stop=(c == kchunks - 1),
                        )
                    for p in range(NPAIR):
                        dst = outs[p][:, ti, :].rearrange(
                            "q (b c) -> q b c", b=2)[:, :, 0:64]
                        src = pm[:, 128 * p:128 * (p + 1)].rearrange(
                            "q (b c) -> q b c", b=2)
                        nc.vector.tensor_copy(dst, src)
                return outs

            out1 = [cst.tile([128, N1], BF, tag=f"out1_{p}", name=f"out1_{p}")
                    for p in range(NPAIR)]
            out2 = [cst.tile([128, N2], BF, tag=f"out2_{p}", name=f"out2_{p}")
                    for p in range(NPAIR)]

            # ---- attention for one 512-query block, one head-pair ----
            def attn_block(qt, kt, ve, outt, qs, p, mtiles):
                qc = slice(QW * qs, QW * (qs + 1))
                pv = [psp.tile([65, QW], F32, tag="pv", name="pv", bufs=4)
                      for _ in range(2)]
                for m in range(mtiles):
                    st = psp.tile([128, 2, QW], F32, tag="st", name="st")
                    for h in range(2):
                        rows = slice(64 * h, 64 * (h + 1))
                        nc.tensor.matmul(
                            st[:, h, :],
                            kt[p][rows, 128 * m:128 * (m + 1)],
                            qt[p][rows, qc],
                            start=True, stop=True,
                        )
                    at = att.tile([128, 2, QW], BF, tag="attn", name="attn")
                    nc.scalar.activation(at, st, Exp)
                    for h in range(2):
                        nc.tensor.matmul(
                            pv[h],
                            ve[p][:, m, 65 * h:65 * (h + 1)],
                            at[:, h, :],
                            start=(m == 0), stop=(m == mtiles - 1),
                        )
                for h in range(2):
                    # 1/den = exp(-ln(den)) on ACT: ~1.2us vs 3.3us DVE
                    # reciprocal; Ln and Exp share one activation table set.
                    rec = sc.tile([128, QW], F32, tag="rec", name="rec")
                    nc.scalar.activation(
                        rec[64:65, :], pv[h][64:65, :],
                        mybir.ActivationFunctionType.Ln)
                    nc.scalar.activation(
                        rec[64:65, :], rec[64:65, :], Exp, scale=-1.0)
                    db = dbp.tile([1, QW], F32, tag="db", name="db")
                    nc.sync.dma_start(out=db, in_=rec[64:65, :])
                    bca = sc.tile([128, QW], F32, tag="bca", name="bca")
                    nc.sync.dma_start(
                        out=bca[0:64, :], in_=_dram_bcast(db[:], 64))
                    if h == 0:
                        nc.vector.tensor_mul(
                            outt[p][0:64, qc],
                            pv[h][0:64, :], bca[0:64, :])
                    else:
                        tmp = sc.tile([64, QW], BF, tag="tmp", name="tmp")
                        nc.vector.tensor_mul(
                            tmp, pv[h][0:64, :], bca[0:64, :])
                        nc.sync.dma_start(
                            out=outt[p][64:128, qc], in_=tmp)

            # ---- per-block output projection (PE gap filler) ----
            def make_oproj(wo, outt, od, cdim):
                def oproj(qb):
                    qc = slice(1024 * qb, 1024 * (qb + 1))
                    for ct in range(cdim // 128):
                        pm = psp.tile([128, 1024], F32, tag="st", name="st")
                        for hc in range(2):
                            for h2 in range(2):
                                nc.tensor.matmul(
                                    pm[:, 512 * h2:512 * (h2 + 1)],
                                    wo[hc][:, 128 * ct:128 * (ct + 1)],
                                    outt[hc][:, 1024 * qb + 512 * h2:
                                             1024 * qb + 512 * (h2 + 1)],
                                    start=(hc == 0), stop=(hc == 1),
                                )
                        ost = sc.tile([128, 1024], F32, tag="ost", name="ost")
                        nc.vector.tensor_copy(ost, pm)
                        nc.sync.dma_start(
                            out=od[:][128 * ct:128 * (ct + 1), qc], in_=ost)
                return oproj

            # dir 1->2 dependencies first, then run attention per block
            k2 = proj_qk(wk2, x2, N2, "k2")
            v2e = proj_vt(wv2, x2, N2, "v2e")
            q1 = proj_qk(wq1, x1, N1, "q1")
            # dir 2->1 projections (scheduler fills gaps with these)
            k1 = proj_qk(wk1, x1, N1, "k1")
            q2 = proj_qk(wq2, x2, N2, "q2")
            v1e = proj_vt(wv1, x1, N1, "v1e")

            oproj1 = make_oproj(wo1, out1, o1p_d, C1)
            oproj2 = make_oproj(wo2, out2, o2p_d, C2)

            for qs in range(N1 // QW):
                for p in range(NPAIR):
                    attn_block(q1, k2, v2e, out1, qs, p, N2 // 128)
                if qs % 2 == 1:
                    oproj1(qs // 2)
            for qs in range(N2 // QW):
                for p in range(NPAIR):
                    attn_block(q2, k1, v1e, out2, qs, p, N1 // 128)
            oproj2(0)

    nc.finalize()
    return nc


def _get_program():
    global _PROGRAM
    if _PROGRAM is None:
        _PROGRAM = _build_program()
    return _PROGRAM


def kernel(seq1, seq2, Wq1, Wk1, Wv1, Wo1, bo1, Wq2, Wk2, Wv2, Wo2, bo2):
    nc = _get_program()
    bf16 = ml_dtypes.bfloat16

    in_maps = []
    for c in range(N_CORES):
        b, g = c // 2, c % 2
        rows = slice(256 * g, 256 * (g + 1))
        in_maps.append({
            "x1": np.ascontiguousarray(
                seq1[b].reshape(C1, N1).astype(bf16)),
            "x2": np.ascontiguousarray(
                seq2[b].reshape(C2, N2).astype(bf16)),
            "wq1": np.ascontiguousarray((Wq1[rows] * SCALE).T.astype(bf16)),
            "wk1": np.ascontiguousarray(Wk1[rows].T.astype(bf16)),
            "wv1": np.ascontiguousarray(Wv1[rows].T.astype(bf16)),
            "wq2": np.ascontiguousarray((Wq2[rows] * SCALE).T.astype(bf16)),
            "wk2": np.ascontiguousarray(Wk2[rows].T.astype(bf16)),
            "wv2": np.ascontiguousarray(Wv2[rows].T.astype(bf16)),
            "wo1": np.ascontiguousarray(Wo1[:, rows].T.astype(bf16)),
            "wo2": np.ascontiguousarray(Wo2[:, rows].T.astype(bf16)),
        })

    res = run_bass_kernel_spmd(nc, in_maps, list(range(N_CORES)),
                               trace=TRACE, tmpdir=TMPDIR)
    LAST["exec_time_ns"] = res.exec_time_ns

    o1 = np.empty((4, C1, 64, 64), np.float32)
    o2 = np.empty((4, C2, 32, 32), np.float32)
    for b in range(4):
        p1 = res.results[2 * b]["o1p"] + res.results[2 * b + 1]["o1p"]
        p2 = res.results[2 * b]["o2p"] + res.results[2 * b + 1]["o2p"]
        o1[b] = (p1 + bo1[:, None]).reshape(C1, 64, 64) + seq1[b]
        o2[b] = (p2 + bo2[:, None]).reshape(C2, 32, 32) + seq2[b]
    return (o1, o2)


# revision 12
# speedup vs baseline: 1.0982x; 1.0982x over previous
"""Bidirectional cross-attention kernel for Trainium2, 8 NeuronCores.

Sharding: core c handles batch b = c//2 and head-group g = c%2 (4 of the 8
heads).  Each core computes q/k/v projections for its heads, both attention
directions (seq1->seq2 and seq2->seq1), and a partial output projection over
its head-group's hid columns.  The host sums the two head-group partials per
batch and adds bias + residual.

Layouts on device (per core):
  x1 [C1=256, N1=4096], x2 [C2=512, N2=1024]  (channels-major, bf16)
  q/k [D-pair=128, N] with head pair (2h, 2h+1) stacked on partitions
  vT_ext [128 kpos-tile, ntiles, 130] = [v_h0 | ones | v_h1 | ones]
  Attention processes 512 queries at a time.  Per key-tile m (128 kpos):
  S^T for both heads of a pair goes into one [128, 2, 512] PSUM tile
  (row-group-paired matmuls at partition bases 0/64), one exp on ACT
  covers both heads -> attn^T bf16 in SBUF, PV matmuls accumulate
  [65, 512] per head (64 v-dims + ones-row = softmax denominator).
  The two head-pairs are interleaved inside the m loop so one pair's
  normalize tail hides under the other pair's matmuls.
  Normalization: reciprocal of the denominator row,
  partition-broadcast via a DRAM bounce, multiply on DVE while copying
  PSUM->SBUF (bf16).  Output projection runs per 512-query block as PE
  gap filler.
"""

import sys

for _p in ("/opt/trn_rl_repo",):
    if _p not in sys.path:
        sys.path.insert(0, _p)

import numpy as np
import ml_dtypes

import concourse.bass as bass
import concourse.tile as tile
from concourse import bacc, mybir
from concourse.bass_utils import run_bass_kernel_spmd

BF = mybir.dt.bfloat16
F32 = mybir.dt.float32
Exp = mybir.ActivationFunctionType.Exp

HEADS = 8
D = 64
SCALE = D ** -0.5
C1, N1 = 256, 4096
C2, N2 = 512, 1024
NPAIR = 2       # head pairs per core
N_CORES = 8
QW = 512        # query block width

TRACE = False
TMPDIR = None
LAST = {}

_PROGRAM = None


def _dram_bcast(ap_1xN, nparts):
    """Partition-broadcast AP for a DRAM source row."""
    return bass.AP(
        tensor=ap_1xN.tensor,
        offset=ap_1xN.offset,
        ap=[[0, nparts]] + [list(x) for x in ap_1xN.ap[1:]],
    )


def _build_program():
    nc = bacc.Bacc("TRN2", target_bir_lowering=False, debug=True)

    x1_d = nc.declare_dram_parameter("x1", [C1, N1], BF, isOutput=False)
    x2_d = nc.declare_dram_parameter("x2", [C2, N2], BF, isOutput=False)
    wq1_d = nc.declare_dram_parameter("wq1", [C1, 256], BF, isOutput=False)
    wk1_d = nc.declare_dram_parameter("wk1", [C1, 256], BF, isOutput=False)
    wv1_d = nc.declare_dram_parameter("wv1", [C1, 256], BF, isOutput=False)
    wq2_d = nc.declare_dram_parameter("wq2", [C2, 256], BF, isOutput=False)
    wk2_d = nc.declare_dram_parameter("wk2", [C2, 256], BF, isOutput=False)
    wv2_d = nc.declare_dram_parameter("wv2", [C2, 256], BF, isOutput=False)
    wo1_d = nc.declare_dram_parameter("wo1", [256, C1], BF, isOutput=False)
    wo2_d = nc.declare_dram_parameter("wo2", [256, C2], BF, isOutput=False)
    o1p_d = nc.declare_dram_parameter("o1p", [C1, N1], F32, isOutput=True)
    o2p_d = nc.declare_dram_parameter("o2p", [C2, N2], F32, isOutput=True)

    with tile.TileContext(nc) as tc:
        with tc.tile_pool(name="const", bufs=1) as cst, \
             tc.tile_pool(name="ps", bufs=2, space="PSUM") as psp, \
             tc.tile_pool(name="att", bufs=8) as att, \
             tc.tile_pool(name="sc", bufs=4) as sc, \
             tc.tile_pool(name="dbounce", bufs=6, space="DRAM") as dbp:

            # ---- load inputs (small x2 first so compute starts early) ----
            x2 = []
            for c in range(4):
                t = cst.tile([128, N2], BF, tag=f"x2_{c}", name=f"x2_{c}")
                nc.sync.dma_start(out=t, in_=x2_d[:][128 * c:128 * (c + 1), :])
                x2.append(t)
            x1 = []
            for c in range(2):
                t = cst.tile([128, N1], BF, tag=f"x1_{c}", name=f"x1_{c}")
                nc.sync.dma_start(out=t, in_=x1_d[:][128 * c:128 * (c + 1), :])
                x1.append(t)

            def load_w(d, kchunks, cols, name):
                ts = []
                for c in range(kchunks):
                    t = cst.tile([128, cols], BF,
                                 tag=f"{name}_{c}", name=f"{name}_{c}")
                    nc.sync.dma_start(out=t, in_=d[:][128 * c:128 * (c + 1), :])
                    ts.append(t)
                return ts

            wk2 = load_w(wk2_d, 4, 256, "wk2")
            wv2 = load_w(wv2_d, 4, 256, "wv2")
            wq1 = load_w(wq1_d, 2, 256, "wq1")
            wk1 = load_w(wk1_d, 2, 256, "wk1")
            wv1 = load_w(wv1_d, 2, 256, "wv1")
            wq2 = load_w(wq2_d, 4, 256, "wq2")
            wo1 = load_w(wo1_d, 2, C1, "wo1")
            wo2 = load_w(wo2_d, 2, C2, "wo2")

            # ---- projections: q/k in [D-pair, N] layout ----
            def proj_qk(wt, xt, n, name):
                outs = []
                kchunks = len(wt)
                for p in range(NPAIR):
                    dst = cst.tile([128, n], BF,
                                   tag=f"{name}_{p}", name=f"{name}_{p}")
                    for nt in range(n // 1024):
                        pm = psp.tile([128, 1024], F32, tag="st", name="st")
                        for c in range(kchunks):
                            for h2 in range(2):
                                nc.tensor.matmul(
                                    pm[:, 512 * h2:512 * (h2 + 1)],
                                    wt[c][:, 128 * p:128 * (p + 1)],
                                    xt[c][:, 1024 * nt + 512 * h2:
                                          1024 * nt + 512 * (h2 + 1)],
                                    start=(c == 0), stop=(c == kchunks - 1),
                                )
                        nc.vector.tensor_copy(
                            dst[:, 1024 * nt:1024 * (nt + 1)], pm)
                    outs.append(dst)
                return outs

            # ---- projections: v^T extended with ones column ----
            def proj_vt(wt, xt, n, name):
                ntiles = n // 128
                kchunks = len(wt)
                outs = []
                for p in range(NPAIR):
                    t = cst.tile([128, ntiles, 130], BF,
                                 tag=f"{name}_{p}", name=f"{name}_{p}")
                    nc.vector.memset(t[:, :, 64:65], 1.0)
                    nc.vector.memset(t[:, :, 129:130], 1.0)
                    outs.append(t)
                for ti in range(ntiles):
                    pm = psp.tile([128, 256], F32, tag="pv", name="pv", bufs=4)
                    for c in range(kchunks):
                        nc.tensor.matmul(
                            pm, xt[c][:, 128 * ti:128 * (ti + 1)], wt[c],
                            start=(c == 0), stop=(c == kchunks - 1),
                        )
                    for p in range(NPAIR):
                        dst = outs[p][:, ti, :].rearrange(
                            "q (b c) -> q b c", b=2)[:, :, 0:64]
                        src = pm[:, 128 * p:128 * (p + 1)].rearrange(
                            "q (b c) -> q b c", b=2)
                        nc.vector.tensor_copy(dst, src)
                return outs

            out1 = [cst.tile([128, N1], BF, tag=f"out1_{p}", name=f"out1_{p}")
                    for p in range(NPAIR)]
            out2 = [cst.tile([128, N2], BF, tag=f"out2_{p}", name=f"out2_{p}")
                    for p in range(NPAIR)]

            # ---- attention for one 512-query block, one head-pair ----
            def attn_block(qt, kt, ve, outt, qs, p, mtiles):
                qc = slice(QW * qs, QW * (qs + 1))
                pv = [psp.tile([65, QW], F32, tag="pv", name="pv", bufs=4)
                      for _ in range(2)]
                for m in range(mtiles):
                    st = psp.tile([128, 2, QW], F32, tag="st", name="st")
                    for h in range(2):
                        rows = slice(64 * h, 64 * (h + 1))
                        nc.tensor.matmul(
                            st[:, h, :],
                            kt[p][rows, 128 * m:128 * (m + 1)],
                            qt[p][rows, qc],
                            start=True, stop=True,
                        )
                    at = att.tile([128, 2, QW], BF, tag="attn", name="attn")
                    nc.scalar.activation(at, st, Exp)
                    for h in range(2):
                        nc.tensor.matmul(
                            pv[h],
                            ve[p][:, m, 65 * h:65 * (h + 1)],
                            at[:, h, :],
                            start=(m == 0), stop=(m == mtiles - 1),
                        )
                for h in range(2):
                    rec = sc.tile([128, QW], F32, tag="rec", name="rec")
                    nc.vector.reciprocal(
                        out=rec[64:65, :], in_=pv[h][64:65, :])
                    db = dbp.tile([1, QW], F32, tag="db", name="db")
                    nc.sync.dma_start(out=db, in_=rec[64:65, :])
                    bca = sc.tile([128, QW], F32, tag="bca", name="bca")
                    nc.sync.dma_start(
                        out=bca[0:64, :], in_=_dram_bcast(db[:], 64))
                    if h == 0:
                        nc.vector.tensor_mul(
                            outt[p][0:64, qc],
                            pv[h][0:64, :], bca[0:64, :])
                    else:
                        tmp = sc.tile([64, QW], BF, tag="tmp", name="tmp")
                        nc.vector.tensor_mul(
                            tmp, pv[h][0:64, :], bca[0:64, :])
                        nc.sync.dma_start(
                            out=outt[p][64:128, qc], in_=tmp)

            # ---- per-block output projection (PE gap filler) ----
            def make_oproj(wo, outt, od, cdim):
                def oproj(qb):
                    qc = slice(1024 * qb, 1024 * (qb + 1))
                    for ct in range(cdim // 128):
                        pm = psp.tile([128, 1024], F32, tag="st", name="st")
                        for hc in range(2):
                            for h2 in range(2):
                                nc.tensor.matmul(
                                    pm[:, 512 * h2:512 * (h2 + 1)],
                                    wo[hc][:, 128 * ct:128 * (ct + 1)],
                                    outt[hc][:, 1024 * qb + 512 * h2:
                                             1024 * qb + 512 * (h2 + 1)],
                                    start=(hc == 0), stop=(hc == 1),
                                )
                        ost = sc.tile([128, 1024], F32, tag="ost", name="ost")
                        nc.vector.tensor_copy(ost, pm)
                        nc.sync.dma_start(
                            out=od[:][128 * ct:128 * (ct + 1), qc], in_=ost)
                return oproj

            # dir 1->2 dependencies first, then run attention per block
            k2 = proj_qk(wk2, x2, N2, "k2")
            v2e = proj_vt(wv2, x2, N2, "v2e")
            q1 = proj_qk(wq1, x1, N1, "q1")
            # dir 2->1 projections (scheduler fills gaps with these)
            k1 = proj_qk(wk1, x1, N1, "k1")
            q2 = proj_qk(wq2, x2, N2, "q2")
            v1e = proj_vt(wv1, x1, N1, "v1e")

            oproj1 = make_oproj(wo1, out1, o1p_d, C1)
            oproj2 = make_oproj(wo2, out2, o2p_d, C2)

            for qs in range(N1 // QW):
                for p in range(NPAIR):
                    attn_block(q1, k2, v2e, out1, qs, p, N2 // 128)
                if qs % 2 == 1:
                    oproj1(qs // 2)
            for qs in range(N2 // QW):
                for p in range(NPAIR):
                    attn_block(q2, k1, v1e, out2, qs, p, N1 // 128)
            oproj2(0)

    nc.finalize()
    return nc


def _get_program():
    global _PROGRAM
    if _PROGRAM is None:
        _PROGRAM = _build_program()
    return _PROGRAM


def kernel(seq1, seq2, Wq1, Wk1, Wv1, Wo1, bo1, Wq2, Wk2, Wv2, Wo2, bo2):
    nc = _get_program()
    bf16 = ml_dtypes.bfloat16

    in_maps = []
    for c in range(N_CORES):
        b, g = c // 2, c % 2
        rows = slice(256 * g, 256 * (g + 1))
        in_maps.append({
            "x1": np.ascontiguousarray(
                seq1[b].reshape(C1, N1).astype(bf16)),
            "x2": np.ascontiguousarray(
                seq2[b].reshape(C2, N2).astype(bf16)),
            "wq1": np.ascontiguousarray((Wq1[rows] * SCALE).T.astype(bf16)),
            "wk1": np.ascontiguousarray(Wk1[rows].T.astype(bf16)),
            "wv1": np.ascontiguousarray(Wv1[rows].T.astype(bf16)),
            "wq2": np.ascontiguousarray((Wq2[rows] * SCALE).T.astype(bf16)),
            "wk2": np.ascontiguousarray(Wk2[rows].T.astype(bf16)),
            "wv2": np.ascontiguousarray(Wv2[rows].T.astype(bf16)),
            "wo1": np.ascontiguousarray(Wo1[:, rows].T.astype(bf16)),
            "wo2": np.ascontiguousarray(Wo2[:, rows].T.astype(bf16)),
        })

    res = run_bass_kernel_spmd(nc, in_maps, list(range(N_CORES)),
                               trace=TRACE, tmpdir=TMPDIR)
    LAST["exec_time_ns"] = res.exec_time_ns

    o1 = np.empty((4, C1, 64, 64), np.float32)
    o2 = np.empty((4, C2, 32, 32), np.float32)
    for b in range(4):
        p1 = res.results[2 * b]["o1p"] + res.results[2 * b + 1]["o1p"]
        p2 = res.results[2 * b]["o2p"] + res.results[2 * b + 1]["o2p"]
        o1[b] = (p1 + bo1[:, None]).reshape(C1, 64, 64) + seq1[b]
        o2[b] = (p2 + bo2[:, None]).reshape(C2, 32, 32) + seq2[b]
    return (o1, o2)
